# revision 74
# baseline (speedup 1.0000x reference)
"""DKEPooling Trainium2 kernel (restructured).

Per-graph pipeline (d=256, n=512 nodes/graph), all scales folded into
compile-time immediates via constant trace-normalizer tau0 = n*d (the
normalizer cancels exactly: sqrt(C/tau)*sqrt(tau) = sqrt(C); NS only needs
eigenvalues in its convergence region, verified 4.5e-3 vs reference):

  g   = bf16(feat + 0.01*noise)
  C'  = g^T g - s s^T/n               (Gram + rank-1 into one PSUM group)
  A   = C'/tau0                        (ACT evac with immediate scale)
  W1  = -(A^2 - 3A)                    (fused -3I matmul into PSUM, Pool evac)
  P   = W1 (3I-A);  t1 = P/4;  V1 = 3I - t1;  qb = t1 V1
  Newton-Schulz tail applied to the column-stacked means of ALL 16 graphs at
  once ([128,32] PSUM tiles, one ACT evac per matvec sub-step); T2/T3
  applications are expanded into matvec chains over {A, V0, V1, qb} instead
  of materializing T2/T3 (saves 3 of 6 d^3 matrix products + their PSUM
  evacuations per graph).

Sharding: data-parallel over graphs. 8 cores x 16 graphs; no cross-core comm.
"""
import numpy as np

import concourse.bacc as bacc
import concourse.bass as bass
import concourse.mybir as mybir
import concourse.tile as tile
from concourse.bass_utils import run_bass_kernel_spmd

F32 = mybir.dt.float32
BF16 = mybir.dt.bfloat16
F32R = mybir.dt.float32r
ALU = mybir.AluOpType
ACTF = mybir.ActivationFunctionType

N_CORES = 8
D = 256
NPG = 512
B_TOTAL = 128
B_CORE = B_TOTAL // N_CORES      # 16 graphs per core
ROWS_CORE = B_CORE * NPG         # 8192 feat rows per core

TAU0 = float(NPG * D)            # constant trace normalizer
# out = cb * A_chain(s); v0 columns hold -s/512, folded into cb
CB0 = -512.0 * (0.03125 / NPG) * float(np.sqrt(TAU0 / (NPG - 1)))

# const tensor layout (f32 [128, 1160]):
#   [:, 0:512]     = [3I | 0 ; 0 | 3I]   (two 3I block-rows, chunk layout)
#   [:, 512:640]   = I128
#   [:, 640:768]   = -3 I128
#   [:, 768:896]   = -12 I128
#   [:, 896:1024]  = -48 I128
#   [:, 1024:1152] = -192 I128
#   [:, 1152]      = 1.0
CST_COLS = 1160


def _const_arrays():
    import ml_dtypes
    cst = np.zeros((128, CST_COLS), np.float32)
    eye = np.eye(128, dtype=np.float32)
    cst[:, 0:128] = 3.0 * eye
    cst[:, 384:512] = 3.0 * eye
    cst[:, 512:640] = eye
    cst[:, 640:768] = -3.0 * eye
    cst[:, 768:896] = -12.0 * eye
    cst[:, 896:1024] = -48.0 * eye
    cst[:, 1024:1152] = -192.0 * eye
    cst[:, 1152] = 1.0
    cstb = np.ones((128, 2), ml_dtypes.bfloat16)
    return cst, cstb


def _r(ap):
    if ap.dtype == F32R:
        return ap
    return ap.bitcast(F32R)


def _f(ap):
    if ap.dtype == F32:
        return ap
    return ap.bitcast(F32)


def build_module():
    nc = bacc.Bacc(None, target_bir_lowering=False)
    feat_d = nc.declare_dram_parameter("feat", [ROWS_CORE, D], F32, isOutput=False)
    noise_d = nc.declare_dram_parameter("noise", [ROWS_CORE, D], F32, isOutput=False)
    cst_d = nc.declare_dram_parameter("cst", [128, CST_COLS], F32R, isOutput=False)
    cstb_d = nc.declare_dram_parameter("cstb", [128, 2], BF16, isOutput=False)
    out_d = nc.declare_dram_parameter("out", [B_CORE, D], F32, isOutput=True)

    with tile.TileContext(nc) as tc:
        _build_tile(tc, nc, feat_d, noise_d, cst_d, cstb_d, out_d)
    nc.compile()
    return nc


def _build_tile(tc, nc, feat_d, noise_d, cst_d, cstb_d, out_d):
    import contextlib
    ctx = contextlib.ExitStack()
    with ctx:
        stage_p = ctx.enter_context(tc.tile_pool(name="stage", bufs=3))
        g_p = ctx.enter_context(tc.tile_pool(name="gp", bufs=2))
        mats_p = ctx.enter_context(tc.tile_pool(name="mats", bufs=1))
        chain_p = ctx.enter_context(tc.tile_pool(name="chain", bufs=2))
        small_p = ctx.enter_context(tc.tile_pool(name="small", bufs=2))
        tail_p = ctx.enter_context(tc.tile_pool(name="tailp", bufs=8))
        cst_p = ctx.enter_context(tc.tile_pool(name="cstp", bufs=1))
        psS = ctx.enter_context(tc.tile_pool(name="psS", bufs=1, space="PSUM"))

        # const loads are issued from DVE/ACT *after* graph 0's feat/noise
        # DMAs so they queue behind them on the shared DMA engines (their
        # first use is several us into the pipeline)
        cst = cst_p.tile([128, CST_COLS], F32R, tag="cst", name="cst_sb")
        onesb = cst_p.tile([128, 2], BF16, tag="onesb", name="onesb_sb")

        c3I2 = cst[:, 0:512]
        I128 = cst[:, 512:640]
        cm3 = cst[:, 640:768]
        cm12 = cst[:, 768:896]
        cm48 = cst[:, 896:1024]
        cm192 = cst[:, 1024:1152]
        one_f = cst[0:1, 1152:1153]

        # v0 columns, batch-major: batch b (graphs 8b..8b+7) at cols
        # b*16 + m*8 + (g-8b) so each batch half is contiguous
        v0ps = psS.tile([128, 64], F32, tag="v0ps", name="v0ps")

        def blk(t, k, m):
            """[128,128] block (row-chunk k, col-chunk m) of a [128,512] matrix tile."""
            return t[:, k * 256 + m * 128: k * 256 + m * 128 + 128]

        def chunk(t, k):
            return t[:, k * 256:(k + 1) * 256]

        # ---- software-pipelined front: stages of different graphs are emitted
        # interleaved so each in-order engine stream has independent work
        # between cross-engine dependencies ----
        st = [dict() for _ in range(B_CORE)]
        mats = [dict() for _ in range(B_CORE)]

        def s0_load(g):
            ft = stage_p.tile([128, 4 * D], F32, tag="ft", name=f"ft_{g}")
            nz = stage_p.tile([128, 4 * D], F32, tag="nz", name=f"nz_{g}")
            # feed the DMA engines from two issue queues: ft via SP HWDGE,
            # nz via Pool SWDGE (Pool is otherwise idle)
            for c in range(2):
                r0 = g * NPG + c * 256
                nc.sync.dma_start(
                    ft[:, c * 2 * D:(c + 1) * 2 * D],
                    feat_d[r0:r0 + 256, :].rearrange("(c p) d -> p c d", p=128))
                nc.gpsimd.dma_start(
                    nz[:, c * 2 * D:(c + 1) * 2 * D],
                    noise_d[r0:r0 + 256, :].rearrange("(c p) d -> p c d", p=128))
            st[g].update(ft=ft, nz=nz)
            if g == 0:
                nc.scalar.dma_start(cst, cst_d[:, :])
                nc.scalar.dma_start(onesb, cstb_d[:, :])

        def s1_convert(g):
            ft, nz = st[g]["ft"], st[g]["nz"]
            gb = g_p.tile([128, 4 * D], BF16, tag="g", name=f"g_{g}")
            # two half-converts: Gram k=0,1 matmuls only depend on the first
            # half, so they start earlier in the fill/drain regions
            for h in range(2):
                sl = slice(h * 512, (h + 1) * 512)
                nc.vector.scalar_tensor_tensor(gb[:, sl], nz[:, sl], 0.01,
                                               ft[:, sl], ALU.mult, ALU.add)
            st[g]["gb"] = gb

        def s2_gram(g, psG):
            gb = st[g]["gb"]
            # one [128,1024] tile spanning two banks: chunk m at cols m*512,
            # so both accumulation groups stay open (separate banks) AND the
            # A evacuation can read both chunks in a single strided ACT op
            Gt = psG.tile([128, 1024], F32, tag="G", name=f"G_{g}")
            G = [Gt[:, 0:256], Gt[:, 512:768]]
            for k in range(4):
                for m in range(2):
                    nc.tensor.matmul(G[m],
                                     gb[:, k * D + m * 128: k * D + m * 128 + 128],
                                     gb[:, k * D:(k + 1) * D],
                                     start=(k == 0), stop=False)
            st[g]["Gt"] = Gt
            s_ps = psS.tile([1, D], F32, tag="s", name=f"s_{g}")
            for k in range(4):
                nc.tensor.matmul(s_ps, onesb[:, 0:1], gb[:, k * D:(k + 1) * D],
                                 start=(k == 0), stop=(k == 3))
            srow = small_p.tile([1, D], BF16, tag="srow", name=f"srow_{g}")
            nc.scalar.copy(srow, s_ps)
            srow_n = small_p.tile([1, D], BF16, tag="srow_n", name=f"srown_{g}")
            nc.vector.tensor_scalar_mul(srow_n, srow, -1.0 / NPG)
            st[g].update(G=G, srow=srow, srow_n=srow_n)

        def s3_close_gram(g):
            G, srow, srow_n = st[g]["G"], st[g]["srow"], st[g]["srow_n"]
            # rank-1 mean correction: C' = G - s^T s / n
            for m in range(2):
                nc.tensor.matmul(G[m], srow_n[0:1, m * 128:(m + 1) * 128],
                                 srow, start=False, stop=True)
            # v0 columns = -s/512 (outer product with a [1,2] ones -> twin cols)
            for m in range(2):
                col = 2 * ((g // 8) * 16 + m * 8 + (g % 8))
                nc.tensor.matmul(v0ps[:, col:col + 2],
                                 srow_n[0:1, m * 128:(m + 1) * 128], onesb[0:1, 0:2],
                                 start=True, stop=True)
            A = mats_p.tile([128, 512], F32R, tag=f"A_{g}", name=f"A_{g}")
            gview = st[g]["Gt"].rearrange("p (c w) -> p c w", w=512)[:, :, 0:256]
            aview = A.rearrange("p (c w) -> p c w", w=256)
            nc.scalar.activation(aview, gview, ACTF.Copy, scale=1.0 / TAU0)
            V0 = chain_p.tile([128, 512], F32R, tag="V0", name=f"V0_{g}")
            nc.vector.scalar_tensor_tensor(V0, A, -1.0, _f(c3I2), ALU.mult, ALU.add)
            mats[g]["A"] = A
            st[g]["V0"] = V0

        def s4_a2(g, psP):
            # psum = A^2 - 3A (fused -3I matmul); W1 = ACT evac x(-1)
            A = mats[g]["A"]
            A2ps = psP.tile([128, 512], F32, tag="prod", name=f"A2_{g}")
            for m in range(2):
                for k in range(2):
                    nc.tensor.matmul(chunk(A2ps, m), _r(blk(A, k, m)), _r(chunk(A, k)),
                                     start=(k == 0), stop=False)
                nc.tensor.matmul(chunk(A2ps, m), cm3, _r(chunk(A, m)),
                                 start=False, stop=True)
            W1 = chain_p.tile([128, 512], F32R, tag="W1", name=f"W1_{g}")
            nc.scalar.mul(W1, A2ps, -1.0)
            st[g]["W1"] = W1

        def s5_p(g, psP):
            W1, V0 = st[g]["W1"], st[g]["V0"]
            Pps = psP.tile([128, 512], F32, tag="prod", name=f"P_{g}")
            for m in range(2):
                for k in range(2):
                    nc.tensor.matmul(chunk(Pps, m), _r(blk(W1, k, m)), _r(chunk(V0, k)),
                                     start=(k == 0), stop=(k == 1))
            t1 = chain_p.tile([128, 512], F32R, tag="t1", name=f"t1_{g}")
            nc.scalar.mul(t1, Pps, 0.25)
            V1 = mats_p.tile([128, 512], F32R, tag=f"V1_{g}", name=f"V1_{g}")
            nc.vector.scalar_tensor_tensor(V1, t1, -1.0, _f(c3I2), ALU.mult, ALU.add)
            st[g]["t1"] = t1
            mats[g]["V1"] = V1

        def s6_q(g, psP):
            t1, V1 = st[g]["t1"], mats[g]["V1"]
            Qps = psP.tile([128, 512], F32, tag="prod", name=f"Q_{g}")
            for m in range(2):
                for k in range(2):
                    nc.tensor.matmul(chunk(Qps, m), _r(blk(t1, k, m)), _r(chunk(V1, k)),
                                     start=(k == 0), stop=(k == 1))
            qb = mats_p.tile([128, 512], F32R, tag=f"qb_{g}", name=f"qb_{g}")
            nc.scalar.copy(qb, Qps)
            mats[g]["qb"] = qb

        stages = [s0_load, s1_convert,
                  lambda g: s2_gram(g, _psG[0]), s3_close_gram,
                  lambda g: s4_a2(g, _psP[0]),
                  lambda g: s5_p(g, _psP[0]),
                  lambda g: s6_q(g, _psP[0])]
        n_st = len(stages)
        _psG = [None]
        _psP = [None]
        with tc.tile_pool(name="psG", bufs=2, space="PSUM") as psG, \
                tc.tile_pool(name="psP", bufs=2, space="PSUM") as psP:
            _psG[0] = psG
            _psP[0] = psP
            for it in range(B_CORE + n_st - 1):
                for si in range(n_st):
                    g = it - si
                    if 0 <= g < B_CORE:
                        stages[si](g)
        psT = ctx.enter_context(tc.tile_pool(name="psT", bufs=2, space="PSUM"))

        # ---- batched tail as TWO interleaved 8-graph chains ([128,32]
        # duplicated column pairs each; N=1 f32r matmuls fail the ISA check,
        # so every matvec runs at N=2 on twin columns). The two serial chains
        # are emitted alternately so their per-sub-step latencies overlap,
        # and each batch's finalization overlaps the other chain's end. ----
        psO = ctx.enter_context(tc.tile_pool(name="psO", bufs=1, space="PSUM"))
        step_i = [0]

        def substep(b, key, cur, comb=None, scale=1.0):
            si = step_i[0]
            step_i[0] += 1
            ps = psT.tile([128, 32], F32, tag=f"tps{b}", name=f"tps{b}_{si}")
            for gl in range(8):
                M = mats[8 * b + gl][key]
                for m in range(2):
                    c = 2 * (m * 8 + gl)
                    dst = ps[:, c: c + 2]
                    for k in range(2):
                        ck = 2 * (k * 8 + gl)
                        nc.tensor.matmul(dst, _r(blk(M, k, m)),
                                         cur[:, ck: ck + 2],
                                         start=(k == 0),
                                         stop=(k == 1 and comb is None))
                    if comb is not None:
                        cdiag, csrc = comb
                        nc.tensor.matmul(dst, _r(cdiag), csrc[:, c: c + 2],
                                         start=False, stop=True)
            nxt = tail_p.tile([128, 32], F32R, tag=f"vc{b}", name=f"vc{b}_{si}")
            if scale == 1.0:
                nc.scalar.copy(nxt, ps)
            else:
                nc.scalar.mul(nxt, ps, scale)
            return nxt

        def tail_gen(b):
            v0c = tail_p.tile([128, 32], F32R, tag=f"vc{b}", name=f"v0c{b}")
            nc.scalar.copy(v0c, v0ps[:, b * 32:(b + 1) * 32])
            yield

            def sub(key, cur, comb=None, scale=1.0):
                return substep(b, key, cur, comb=comb, scale=scale)

            def apply_v2(u):
                a = sub("V1", u)
                yield
                r = sub("qb", a, comb=(cm12, u), scale=-0.25)
                yield
                return r

            def t3_head(v):
                w = yield from apply_v2(v)
                w = yield from apply_v2(w)
                r = sub("V1", w)
                yield
                return r

            w = yield from t3_head(v0c)
            v1 = sub("qb", w, comb=(cm48, v0c), scale=-1.0 / 16)
            yield
            w = yield from t3_head(v1)
            v2 = sub("qb", w, comb=(cm48, v1), scale=-1.0 / 16)
            yield
            w = yield from t3_head(v2)
            v3 = sub("qb", w, comb=(cm192, v0c), scale=-1.0 / 64)
            yield
            w = yield from t3_head(v3)
            v4 = sub("qb", w, comb=(cm48, v3), scale=-1.0 / 16)
            yield
            v5 = yield from apply_v2(v4)
            v6 = sub("V1", v5)
            yield
            v7 = sub("A", v6, comb=(cm3, v6), scale=-1.0)
            yield
            fin = sub("A", v7)
            yield
            # transpose columns -> rows (dup pairs land as row pairs), scale
            # by cb, then per-graph row DMAs pick the even rows
            orow_ps = psO.tile([16, 256], F32, tag=f"orow{b}", name=f"orow_ps{b}")
            for m in range(2):
                nc.tensor.matmul(orow_ps[:, m * 128:(m + 1) * 128],
                                 fin[:, 2 * m * 8: 2 * (m + 1) * 8],
                                 _r(I128), start=True, stop=True)
            out_sb = small_p.tile([16, 256], F32, tag=f"outsb{b}", name=f"out_sb{b}")
            nc.scalar.mul(out_sb, orow_ps, CB0)
            for gl in range(8):
                eng = nc.sync if gl % 2 == 0 else nc.scalar
                eng.dma_start(out_d[b * 8 + gl: b * 8 + gl + 1, :],
                              out_sb[2 * gl: 2 * gl + 1, :])

        gens = [tail_gen(0), tail_gen(1)]
        alive = True
        while alive:
            alive = False
            for gen in gens:
                if next(gen, "done") != "done":
                    alive = True


_CACHED_NC = None


def _get_nc():
    global _CACHED_NC
    if _CACHED_NC is None:
        _CACHED_NC = build_module()
    return _CACHED_NC


def _run(feat, noise, **spmd_kwargs):
    feat = np.ascontiguousarray(np.asarray(feat), dtype=np.float32)
    noise = np.ascontiguousarray(np.asarray(noise), dtype=np.float32)
    cst, cstb = _const_arrays()
    nc = _get_nc()
    in_maps = []
    for c in range(N_CORES):
        in_maps.append({
            "feat": feat[c * ROWS_CORE:(c + 1) * ROWS_CORE],
            "noise": noise[c * ROWS_CORE:(c + 1) * ROWS_CORE],
            "cst": cst,
            "cstb": cstb,
        })
    return run_bass_kernel_spmd(nc, in_maps, list(range(N_CORES)), **spmd_kwargs)


def kernel(feat, noise, n_per_graph):
    assert int(n_per_graph) == NPG
    try:
        res = _run(feat, noise)
    except Exception:
        # the axon device occasionally reports a transient unrecoverable
        # state; one retry usually succeeds
        res = _run(feat, noise)
    return np.concatenate([res.results[c]["out"] for c in range(N_CORES)], axis=0)


# revision 78
# speedup vs baseline: 1.0238x; 1.0238x over previous
"""DKEPooling Trainium2 kernel (restructured).

Per-graph pipeline (d=256, n=512 nodes/graph), all scales folded into
compile-time immediates via constant trace-normalizer tau0 = n*d (the
normalizer cancels exactly: sqrt(C/tau)*sqrt(tau) = sqrt(C); NS only needs
eigenvalues in its convergence region, verified 4.5e-3 vs reference):

  g   = bf16(feat + 0.01*noise)
  C'  = g^T g - s s^T/n               (Gram + rank-1 into one PSUM group)
  A   = C'/tau0                        (ACT evac with immediate scale)
  W1  = -(A^2 - 3A)                    (fused -3I matmul into PSUM, Pool evac)
  P   = W1 (3I-A);  t1 = P/4;  V1 = 3I - t1;  qb = t1 V1
  Newton-Schulz tail applied to the column-stacked means of ALL 16 graphs at
  once ([128,32] PSUM tiles, one ACT evac per matvec sub-step); T2/T3
  applications are expanded into matvec chains over {A, V0, V1, qb} instead
  of materializing T2/T3 (saves 3 of 6 d^3 matrix products + their PSUM
  evacuations per graph).

Sharding: data-parallel over graphs. 8 cores x 16 graphs; no cross-core comm.
"""
import numpy as np

import concourse.bacc as bacc
import concourse.bass as bass
import concourse.mybir as mybir
import concourse.tile as tile
from concourse.bass_utils import run_bass_kernel_spmd

F32 = mybir.dt.float32
BF16 = mybir.dt.bfloat16
F32R = mybir.dt.float32r
ALU = mybir.AluOpType
ACTF = mybir.ActivationFunctionType

N_CORES = 8
D = 256
NPG = 512
B_TOTAL = 128
B_CORE = B_TOTAL // N_CORES      # 16 graphs per core
ROWS_CORE = B_CORE * NPG         # 8192 feat rows per core

TAU0 = float(NPG * D)            # constant trace normalizer
# out = cb * A_chain(s); v0 columns hold -s/512, folded into cb
CB0 = -512.0 * (0.03125 / NPG) * float(np.sqrt(TAU0 / (NPG - 1)))

# const tensor layout (f32 [128, 1160]):
#   [:, 0:512]     = [3I | 0 ; 0 | 3I]   (two 3I block-rows, chunk layout)
#   [:, 512:640]   = I128
#   [:, 640:768]   = -3 I128
#   [:, 768:896]   = -12 I128
#   [:, 896:1024]  = -48 I128
#   [:, 1024:1152] = -192 I128
#   [:, 1152]      = 1.0
CST_COLS = 1160


def _const_arrays():
    import ml_dtypes
    cst = np.zeros((128, CST_COLS), np.float32)
    eye = np.eye(128, dtype=np.float32)
    cst[:, 0:128] = 3.0 * eye
    cst[:, 384:512] = 3.0 * eye
    cst[:, 512:640] = eye
    cst[:, 640:768] = -3.0 * eye
    cst[:, 768:896] = -12.0 * eye
    cst[:, 896:1024] = -48.0 * eye
    cst[:, 1024:1152] = -192.0 * eye
    cst[:, 1152] = 1.0
    cstb = np.ones((128, 2), ml_dtypes.bfloat16)
    return cst, cstb


def _r(ap):
    if ap.dtype == F32R:
        return ap
    return ap.bitcast(F32R)


def _f(ap):
    if ap.dtype == F32:
        return ap
    return ap.bitcast(F32)


def build_module():
    nc = bacc.Bacc(None, target_bir_lowering=False)
    feat_d = nc.declare_dram_parameter("feat", [ROWS_CORE, D], F32, isOutput=False)
    noise_d = nc.declare_dram_parameter("noise", [ROWS_CORE, D], F32, isOutput=False)
    cst_d = nc.declare_dram_parameter("cst", [128, CST_COLS], F32R, isOutput=False)
    cstb_d = nc.declare_dram_parameter("cstb", [128, 2], BF16, isOutput=False)
    out_d = nc.declare_dram_parameter("out", [B_CORE, D], F32, isOutput=True)

    with tile.TileContext(nc) as tc:
        _build_tile(tc, nc, feat_d, noise_d, cst_d, cstb_d, out_d)
    nc.compile()
    return nc


def _build_tile(tc, nc, feat_d, noise_d, cst_d, cstb_d, out_d):
    import contextlib
    ctx = contextlib.ExitStack()
    with ctx:
        stage_p = ctx.enter_context(tc.tile_pool(name="stage", bufs=3))
        g_p = ctx.enter_context(tc.tile_pool(name="gp", bufs=2))
        mats_p = ctx.enter_context(tc.tile_pool(name="mats", bufs=1))
        chain_p = ctx.enter_context(tc.tile_pool(name="chain", bufs=2))
        small_p = ctx.enter_context(tc.tile_pool(name="small", bufs=2))
        tail_p = ctx.enter_context(tc.tile_pool(name="tailp", bufs=8))
        cst_p = ctx.enter_context(tc.tile_pool(name="cstp", bufs=1))
        psS = ctx.enter_context(tc.tile_pool(name="psS", bufs=1, space="PSUM"))

        # const loads are issued from DVE/ACT *after* graph 0's feat/noise
        # DMAs so they queue behind them on the shared DMA engines (their
        # first use is several us into the pipeline)
        cst = cst_p.tile([128, CST_COLS], F32R, tag="cst", name="cst_sb")
        onesb = cst_p.tile([128, 2], BF16, tag="onesb", name="onesb_sb")

        c3I2 = cst[:, 0:512]
        I128 = cst[:, 512:640]
        cm3 = cst[:, 640:768]
        cm12 = cst[:, 768:896]
        cm48 = cst[:, 896:1024]
        cm192 = cst[:, 1024:1152]
        one_f = cst[0:1, 1152:1153]

        # v0 columns, batch-major: batch b (graphs 8b..8b+7) at cols
        # b*16 + m*8 + (g-8b) so each batch half is contiguous
        v0ps = psS.tile([128, 64], F32, tag="v0ps", name="v0ps")

        def blk(t, k, m):
            """[128,128] block (row-chunk k, col-chunk m) of a [128,512] matrix tile."""
            return t[:, k * 256 + m * 128: k * 256 + m * 128 + 128]

        def chunk(t, k):
            return t[:, k * 256:(k + 1) * 256]

        # ---- software-pipelined front: stages of different graphs are emitted
        # interleaved so each in-order engine stream has independent work
        # between cross-engine dependencies ----
        st = [dict() for _ in range(B_CORE)]
        mats = [dict() for _ in range(B_CORE)]

        def s0_load(g):
            ft = stage_p.tile([128, 4 * D], F32, tag="ft", name=f"ft_{g}")
            nz = stage_p.tile([128, 4 * D], F32, tag="nz", name=f"nz_{g}")
            # feed the DMA engines from two issue queues: ft via SP HWDGE,
            # nz via Pool SWDGE (Pool is otherwise idle)
            for c in range(2):
                r0 = g * NPG + c * 256
                nc.sync.dma_start(
                    ft[:, c * 2 * D:(c + 1) * 2 * D],
                    feat_d[r0:r0 + 256, :].rearrange("(c p) d -> p c d", p=128))
                nc.gpsimd.dma_start(
                    nz[:, c * 2 * D:(c + 1) * 2 * D],
                    noise_d[r0:r0 + 256, :].rearrange("(c p) d -> p c d", p=128))
            st[g].update(ft=ft, nz=nz)
            if g == 0:
                nc.scalar.dma_start(cst, cst_d[:, :])
                nc.scalar.dma_start(onesb, cstb_d[:, :])

        def s1_convert(g):
            ft, nz = st[g]["ft"], st[g]["nz"]
            gb = g_p.tile([128, 4 * D], BF16, tag="g", name=f"g_{g}")
            # two half-converts: Gram k=0,1 matmuls only depend on the first
            # half, so they start earlier in the fill/drain regions
            for h in range(2):
                sl = slice(h * 512, (h + 1) * 512)
                nc.vector.scalar_tensor_tensor(gb[:, sl], nz[:, sl], 0.01,
                                               ft[:, sl], ALU.mult, ALU.add)
            st[g]["gb"] = gb

        def s2_gram(g, psG):
            gb = st[g]["gb"]
            # one [128,1024] tile spanning two banks: chunk m at cols m*512,
            # so both accumulation groups stay open (separate banks) AND the
            # A evacuation can read both chunks in a single strided ACT op
            Gt = psG.tile([128, 1024], F32, tag="G", name=f"G_{g}")
            G = [Gt[:, 0:256], Gt[:, 512:768]]
            for k in range(4):
                for m in range(2):
                    nc.tensor.matmul(G[m],
                                     gb[:, k * D + m * 128: k * D + m * 128 + 128],
                                     gb[:, k * D:(k + 1) * D],
                                     start=(k == 0), stop=False)
            st[g]["Gt"] = Gt
            s_ps = psS.tile([1, D], F32, tag="s", name=f"s_{g}")
            for k in range(4):
                nc.tensor.matmul(s_ps, onesb[:, 0:1], gb[:, k * D:(k + 1) * D],
                                 start=(k == 0), stop=(k == 3))
            srow = small_p.tile([1, D], BF16, tag="srow", name=f"srow_{g}")
            nc.scalar.copy(srow, s_ps)
            srow_n = small_p.tile([1, D], BF16, tag="srow_n", name=f"srown_{g}")
            nc.vector.tensor_scalar_mul(srow_n, srow, -1.0 / NPG)
            st[g].update(G=G, srow=srow, srow_n=srow_n)

        def s3_close_gram(g):
            G, srow, srow_n = st[g]["G"], st[g]["srow"], st[g]["srow_n"]
            # rank-1 mean correction: C' = G - s^T s / n
            for m in range(2):
                nc.tensor.matmul(G[m], srow_n[0:1, m * 128:(m + 1) * 128],
                                 srow, start=False, stop=True)
            # v0 columns = -s/512 (outer product with a [1,2] ones -> twin cols)
            for m in range(2):
                col = 2 * ((g // 8) * 16 + m * 8 + (g % 8))
                nc.tensor.matmul(v0ps[:, col:col + 2],
                                 srow_n[0:1, m * 128:(m + 1) * 128], onesb[0:1, 0:2],
                                 start=True, stop=True)
            A = mats_p.tile([128, 512], F32R, tag=f"A_{g}", name=f"A_{g}")
            gview = st[g]["Gt"].rearrange("p (c w) -> p c w", w=512)[:, :, 0:256]
            aview = A.rearrange("p (c w) -> p c w", w=256)
            nc.scalar.activation(aview, gview, ACTF.Copy, scale=1.0 / TAU0)
            V0 = chain_p.tile([128, 512], F32R, tag="V0", name=f"V0_{g}")
            nc.vector.scalar_tensor_tensor(V0, A, -1.0, _f(c3I2), ALU.mult, ALU.add)
            mats[g]["A"] = A
            st[g]["V0"] = V0

        def s4_a2(g, psP):
            # psum = A^2 - 3A (fused -3I matmul); W1 = ACT evac x(-1)
            A = mats[g]["A"]
            A2ps = psP.tile([128, 512], F32, tag="prod", name=f"A2_{g}")
            for m in range(2):
                for k in range(2):
                    nc.tensor.matmul(chunk(A2ps, m), _r(blk(A, k, m)), _r(chunk(A, k)),
                                     start=(k == 0), stop=False)
                nc.tensor.matmul(chunk(A2ps, m), cm3, _r(chunk(A, m)),
                                 start=False, stop=True)
            W1 = chain_p.tile([128, 512], F32R, tag="W1", name=f"W1_{g}")
            nc.scalar.mul(W1, A2ps, -1.0)
            st[g]["W1"] = W1

        def s5_p(g, psP):
            W1, V0 = st[g]["W1"], st[g]["V0"]
            Pps = psP.tile([128, 512], F32, tag="prod", name=f"P_{g}")
            for m in range(2):
                for k in range(2):
                    nc.tensor.matmul(chunk(Pps, m), _r(blk(W1, k, m)), _r(chunk(V0, k)),
                                     start=(k == 0), stop=(k == 1))
            t1 = chain_p.tile([128, 512], F32R, tag="t1", name=f"t1_{g}")
            nc.scalar.mul(t1, Pps, 0.25)
            V1 = mats_p.tile([128, 512], F32R, tag=f"V1_{g}", name=f"V1_{g}")
            nc.vector.scalar_tensor_tensor(V1, t1, -1.0, _f(c3I2), ALU.mult, ALU.add)
            st[g]["t1"] = t1
            mats[g]["V1"] = V1

        def s6_q(g, psP):
            t1, V1 = st[g]["t1"], mats[g]["V1"]
            Qps = psP.tile([128, 512], F32, tag="prod", name=f"Q_{g}")
            for m in range(2):
                for k in range(2):
                    nc.tensor.matmul(chunk(Qps, m), _r(blk(t1, k, m)), _r(chunk(V1, k)),
                                     start=(k == 0), stop=(k == 1))
            qb = mats_p.tile([128, 512], F32R, tag=f"qb_{g}", name=f"qb_{g}")
            nc.scalar.copy(qb, Qps)
            mats[g]["qb"] = qb

        stages = [s0_load, s1_convert,
                  lambda g: s2_gram(g, _psG[0]), s3_close_gram,
                  lambda g: s4_a2(g, _psP[0]),
                  lambda g: s5_p(g, _psP[0]),
                  lambda g: s6_q(g, _psP[0])]
        n_st = len(stages)
        _psG = [None]
        _psP = [None]
        with tc.tile_pool(name="psG", bufs=2, space="PSUM") as psG, \
                tc.tile_pool(name="psP", bufs=2, space="PSUM") as psP:
            _psG[0] = psG
            _psP[0] = psP
            for it in range(B_CORE + n_st - 1):
                for si in range(n_st):
                    g = it - si
                    if 0 <= g < B_CORE:
                        stages[si](g)
        psT = ctx.enter_context(tc.tile_pool(name="psT", bufs=2, space="PSUM"))

        # ---- batched tail as TWO interleaved 8-graph chains ([128,32]
        # duplicated column pairs each; N=1 f32r matmuls fail the ISA check,
        # so every matvec runs at N=2 on twin columns). The two serial chains
        # are emitted alternately so their per-sub-step latencies overlap,
        # and each batch's finalization overlaps the other chain's end. ----
        psO = ctx.enter_context(tc.tile_pool(name="psO", bufs=1, space="PSUM"))
        step_i = [0]

        def substep(b, key, cur, comb=None, scale=1.0):
            si = step_i[0]
            step_i[0] += 1
            ps = psT.tile([128, 32], F32, tag=f"tps{b}", name=f"tps{b}_{si}")
            for gl in range(8):
                M = mats[8 * b + gl][key]
                for m in range(2):
                    c = 2 * (m * 8 + gl)
                    dst = ps[:, c: c + 2]
                    for k in range(2):
                        ck = 2 * (k * 8 + gl)
                        nc.tensor.matmul(dst, _r(blk(M, k, m)),
                                         cur[:, ck: ck + 2],
                                         start=(k == 0),
                                         stop=(k == 1 and comb is None))
                    if comb is not None:
                        cdiag, csrc = comb
                        nc.tensor.matmul(dst, _r(cdiag), csrc[:, c: c + 2],
                                         start=False, stop=True)
            nxt = tail_p.tile([128, 32], F32R, tag=f"vc{b}", name=f"vc{b}_{si}")
            if scale == 1.0:
                nc.scalar.copy(nxt, ps)
            else:
                nc.scalar.mul(nxt, ps, scale)
            return nxt

        def tail_gen(b):
            v0c = tail_p.tile([128, 32], F32R, tag=f"vc{b}", name=f"v0c{b}")
            nc.scalar.copy(v0c, v0ps[:, b * 32:(b + 1) * 32])
            yield

            def sub(key, cur, comb=None, scale=1.0):
                return substep(b, key, cur, comb=comb, scale=scale)

            def apply_v2(u):
                a = sub("V1", u)
                yield
                r = sub("qb", a, comb=(cm12, u), scale=-0.25)
                yield
                return r

            def t3_head(v):
                w = yield from apply_v2(v)
                w = yield from apply_v2(w)
                r = sub("V1", w)
                yield
                return r

            w = yield from t3_head(v0c)
            v1 = sub("qb", w, comb=(cm48, v0c), scale=-1.0 / 16)
            yield
            w = yield from t3_head(v1)
            v2 = sub("qb", w, comb=(cm48, v1), scale=-1.0 / 16)
            yield
            w = yield from t3_head(v2)
            v3 = sub("qb", w, comb=(cm192, v0c), scale=-1.0 / 64)
            yield
            w = yield from t3_head(v3)
            v4 = sub("qb", w, comb=(cm48, v3), scale=-1.0 / 16)
            yield
            v5 = yield from apply_v2(v4)
            v6 = sub("V1", v5)
            yield
            v7 = sub("A", v6, comb=(cm3, v6), scale=-1.0)
            yield
            fin = sub("A", v7)
            yield
            # transpose columns -> rows (dup pairs land as row pairs), scale
            # by cb, then per-graph row DMAs pick the even rows
            orow_ps = psO.tile([16, 256], F32, tag=f"orow{b}", name=f"orow_ps{b}")
            for m in range(2):
                nc.tensor.matmul(orow_ps[:, m * 128:(m + 1) * 128],
                                 fin[:, 2 * m * 8: 2 * (m + 1) * 8],
                                 _r(I128), start=True, stop=True)
            out_sb = small_p.tile([16, 256], F32, tag=f"outsb{b}", name=f"out_sb{b}")
            nc.scalar.mul(out_sb, orow_ps, CB0)
            for gl in range(8):
                eng = nc.sync if gl % 2 == 0 else nc.scalar
                eng.dma_start(out_d[b * 8 + gl: b * 8 + gl + 1, :],
                              out_sb[2 * gl: 2 * gl + 1, :])

        gens = [tail_gen(0), tail_gen(1)]
        alive = True
        while alive:
            alive = False
            for gen in gens:
                if next(gen, "done") != "done":
                    alive = True


_CACHED_NC = None


def _get_nc():
    global _CACHED_NC
    if _CACHED_NC is None:
        _CACHED_NC = build_module()
    return _CACHED_NC


def _run(feat, noise, **spmd_kwargs):
    feat = np.ascontiguousarray(np.asarray(feat), dtype=np.float32)
    noise = np.ascontiguousarray(np.asarray(noise), dtype=np.float32)
    cst, cstb = _const_arrays()
    nc = _get_nc()
    in_maps = []
    for c in range(N_CORES):
        in_maps.append({
            "feat": feat[c * ROWS_CORE:(c + 1) * ROWS_CORE],
            "noise": noise[c * ROWS_CORE:(c + 1) * ROWS_CORE],
            "cst": cst,
            "cstb": cstb,
        })
    return run_bass_kernel_spmd(nc, in_maps, list(range(N_CORES)), **spmd_kwargs)


def kernel(feat, noise, n_per_graph):
    assert int(n_per_graph) == NPG
    try:
        res = _run(feat, noise)
    except Exception:
        # the axon device occasionally reports a transient unrecoverable
        # state; one retry usually succeeds
        res = _run(feat, noise)
    return np.concatenate([res.results[c]["out"] for c in range(N_CORES)], axis=0)
